# revision 2
# baseline (speedup 1.0000x reference)
"""DirSageConv Trainium2 kernel v2 (8 NeuronCores, SPMD).

Q7 descriptor generation is the bottleneck (~5ns/idx gather, ~8ns/idx
scatter on the single GpSimd engine). v2 keeps the per-edge dma_gather but
replaces the DRAM scatter-add with segment-sum on the idle TensorEngine:

- x packed as quad-rows x4b [25088, 256] bf16 (4 nodes per 512B row) so a
  single int16 index block covers all 100K nodes (idx = src//4 < 25088).
- Edges sorted by destination, grouped per 128-dst tile, padded to 128
  multiples; per-tile chunk counts equalized across cores (SPMD program).
- Per 128-edge chunk: gather lands [128 edges, 256] bf16; a [128, 512]
  one-hot built on DVE from dsel = (dst_local + 128*(src%4)) - 256 both
  picks the dst column and the src quarter (shift keeps values bf16-exact);
  4 matmuls accumulate meanT [64 feat, 128 dst] into PSUM per dst tile.
- Epilogue per tile: scale by 1/deg (partition-broadcast row), cast bf16;
  per 4 tiles one matmul with W -> ELU -> yT chunk store.
- ELU(z+b) = min(exp(z+b)-1, relu(z+b)): exp on ACT (bias arg), relu+bias
  on DVE (tensor_scalar add/max), min on DVE. ACT runs only Exp (no
  activation-table churn).
- Self branch: per-core x slice, PE transpose -> xT [64, 512], two matmuls
  with ELU between, yT_self.

Host assembles/transposes the three outputs.
"""
import sys

sys.path.insert(0, "/opt/trn_rl_repo")

import ml_dtypes
import numpy as np

import concourse.bacc as bacc
import concourse.bass as bass
import concourse.mybir as mybir
from concourse import tile
from concourse.bass_utils import run_bass_kernel_spmd

F32 = mybir.dt.float32
BF16 = mybir.dt.bfloat16
I16 = mybir.dt.int16

AF = mybir.ActivationFunctionType
ALU = mybir.AluOpType

N = 100000
F_IN = 64
F_OUT = 128
F_HID = 512
NCORES = 8
NPC = N // NCORES              # 12500
NTILE = -(-NPC // 128)         # 98
NPAD = NTILE * 128             # 12544
NQROW = 25088                  # padded quad-rows
QPC = NPC // 4                 # 3125 quad rows per core
QSELF = NPAD // 4              # 3136 quad rows loaded by self branch
TILES_PER_SLICE = 4
CH_MAX = 92


def _wrap_idx16(slots):
    """[SLOTS] int16 -> [128, SLOTS//16] wrapped by 16, replicated 8x."""
    n = slots.shape[0]
    a = slots.reshape(n // 16, 16).T.astype(np.int16)
    return np.tile(a, (8, 1)).copy()


def prep_direction(key, val):
    """key = aggregation target node, val = gathered node (both [E] int64).

    Returns gidx [8, 128, SLOTS/16] i16, dsel [8, 128, CHTOT] f32 (shifted
    by -256), recip [8, 1, NPAD] f32, k_t [NTILE] chunks/tile (shared).
    """
    E = key.shape[0]
    core = np.minimum(key // NPC, NCORES - 1)
    kl = key - core * NPC
    tl = kl // 128
    dl = kl % 128

    cnt = np.zeros((NCORES, NTILE), np.int64)
    np.add.at(cnt, (core, tl), 1)
    k_t = -(-cnt.max(axis=0) // 128)
    k_t = np.maximum(k_t, 1)
    CHTOT = int(k_t.sum())
    SLOTS = CHTOT * 128
    toff = np.concatenate([[0], np.cumsum(k_t * 128)])

    order = np.lexsort((tl, core))
    core_s, tl_s, dl_s, val_s = core[order], tl[order], dl[order], val[order]
    grp = core_s * NTILE + tl_s
    m = np.empty(E, np.bool_)
    m[0] = True
    np.not_equal(grp[1:], grp[:-1], out=m[1:])
    starts = np.flatnonzero(m)
    gid = np.cumsum(m) - 1
    pos = np.arange(E) - starts[gid]

    slot = toff[tl_s] + pos
    gidx = np.zeros((NCORES, SLOTS), np.int16)
    dsel = np.full((NCORES, SLOTS), -257.0, np.float32)
    gidx[core_s, slot] = (val_s // 4).astype(np.int16)
    dsel[core_s, slot] = dl_s + 128.0 * (val_s % 4) - 256.0

    deg = np.bincount(key, minlength=N).astype(np.float32)
    recip = 1.0 / np.maximum(deg, 1.0)
    key_s = key[order]
    rsel = np.zeros((NCORES, SLOTS), np.float32)
    rsel[core_s, slot] = recip[key_s]

    gw = np.stack([_wrap_idx16(gidx[c]) for c in range(NCORES)])
    ds = dsel.reshape(NCORES, CHTOT, 128).transpose(0, 2, 1).copy()
    rs = rsel.reshape(NCORES, CHTOT, 128).transpose(0, 2, 1).copy()
    return gw, ds, rs, k_t


def build_nc(k_t_in, k_t_out):
    nc = bacc.Bacc("TRN2", target_bir_lowering=False, debug=False,
                   enable_asserts=False, dynamic_dma_scratch_size=32768)

    x4b_d = nc.dram_tensor("x4b", [NQROW, 256], BF16, kind="ExternalInput")
    xself_d = nc.dram_tensor("xself", [NPAD, 128], BF16,
                             kind="ExternalInput")
    iota_d = nc.dram_tensor("iota512", [128, 512], BF16, kind="ExternalInput")
    w1_d = nc.dram_tensor("w1", [64, 512], BF16, kind="ExternalInput")
    b1_d = nc.dram_tensor("b1", [128, 4], F32, kind="ExternalInput")
    w2_d = nc.dram_tensor("w2p", [128, 512], BF16, kind="ExternalInput")
    b2_d = nc.dram_tensor("b2", [128, 1], F32, kind="ExternalInput")

    dirs = []
    for name, k_t in (("in", k_t_in), ("out", k_t_out)):
        CHTOT = int(k_t.sum())
        dirs.append(dict(
            name=name, k_t=k_t, CHTOT=CHTOT,
            gidx=nc.dram_tensor(f"gidx_{name}", [128, CHTOT * 8], I16,
                                kind="ExternalInput"),
            dsel=nc.dram_tensor(f"dsel_{name}", [128, CHTOT], F32,
                                kind="ExternalInput"),
            rsel=nc.dram_tensor(f"rsel_{name}", [128, CHTOT], F32,
                                 kind="ExternalInput"),
            w=nc.dram_tensor(f"w_{name}", [64, 128], BF16,
                             kind="ExternalInput"),
            b=nc.dram_tensor(f"b_{name}", [128, 1], F32,
                             kind="ExternalInput"),
            yT=nc.dram_tensor(f"yT_{name}", [128, NPAD], F32,
                              kind="ExternalOutput"),
        ))
    yself_d = nc.dram_tensor("yT_self", [128, NPAD], F32,
                             kind="ExternalOutput")

    with tile.TileContext(nc) as tc:
        with tc.tile_pool(name="const", bufs=1) as cpool, \
             tc.tile_pool(name="idx", bufs=3) as ipool, \
             tc.tile_pool(name="feat", bufs=2) as fpool, \
             tc.tile_pool(name="oh", bufs=4) as opool, \
             tc.tile_pool(name="mt", bufs=2) as mpool, \
             tc.tile_pool(name="ep", bufs=3) as epool, \
             tc.tile_pool(name="selfp", bufs=2) as spool, \
             tc.tile_pool(name="ps", bufs=2, space="PSUM") as pspool, \
             tc.tile_pool(name="psy", bufs=2, space="PSUM") as psypool, \
             tc.tile_pool(name="psh", bufs=2, space="PSUM") as pshpool:

            iota = cpool.tile([128, 512], BF16)
            nc.sync.dma_start(iota[:], iota_d[:])
            w1 = cpool.tile([64, 512], BF16)
            nc.sync.dma_start(w1[:], w1_d[:])
            b1 = cpool.tile([128, 4], F32)
            nc.sync.dma_start(b1[:], b1_d[:])
            w2 = cpool.tile([128, 512], BF16)
            nc.sync.dma_start(w2[:], w2_d[:])
            b2 = cpool.tile([128, 1], F32)
            nc.sync.dma_start(b2[:], b2_d[:])
            wdir, bdir = {}, {}
            for d in dirs:
                wt = cpool.tile([64, 128], BF16, tag="w_" + d["name"])
                nc.sync.dma_start(wt[:], d["w"][:])
                bt = cpool.tile([128, 1], F32, tag="b_" + d["name"])
                nc.sync.dma_start(bt[:], d["b"][:])
                wdir[d["name"]] = wt
                bdir[d["name"]] = bt

            def emit_elu(ps_ap, bias_ap, out_tile, w, pool, tagpfx):
                """out = elu(ps + bias); ACT does exp, DVE does relu+min."""
                e = pool.tile([128, 512], F32, tag=tagpfx + "e")
                nc.scalar.activation(e[:, :w], ps_ap, AF.Exp, bias=bias_ap)
                r = pool.tile([128, 512], F32, tag=tagpfx + "r")
                nc.vector.tensor_scalar(r[:, :w], ps_ap, bias_ap, 0.0,
                                        ALU.add, ALU.max)
                nc.vector.scalar_tensor_tensor(out_tile, e[:, :w], 1.0,
                                               r[:, :w], ALU.subtract,
                                               ALU.min)

            # ---------------- directions ----------------
            for d in dirs:
                k_t = d["k_t"]
                name = d["name"]
                mt4 = None
                t0 = 0
                while t0 < NTILE:
                    t1 = min(t0 + TILES_PER_SLICE, NTILE)
                    n0 = int(k_t[:t0].sum())
                    n1 = int(k_t[:t1].sum())
                    CH = n1 - n0
                    assert CH <= CH_MAX, (t0, CH)
                    gi = ipool.tile([128, CH_MAX * 8], I16, tag="gi")
                    nc.sync.dma_start(gi[:, :CH * 8],
                                      d["gidx"][:, n0 * 8:n1 * 8])
                    ds = ipool.tile([128, CH_MAX], F32, tag="ds")
                    nc.sync.dma_start(ds[:, :CH], d["dsel"][:, n0:n1])
                    rs = ipool.tile([128, CH_MAX], F32, tag="rs")
                    nc.sync.dma_start(rs[:, :CH], d["rsel"][:, n0:n1])
                    feat = fpool.tile([128, CH_MAX, 256], BF16, tag="feat")
                    nc.gpsimd.dma_gather(feat[:, :CH, :], x4b_d[:],
                                         gi[:, :CH * 8], CH * 128, CH * 128,
                                         256, single_packet=False)
                    n = n0
                    for t in range(t0, t1):
                        ps = pspool.tile([64, 128], F32, tag="agg")
                        kt = int(k_t[t])
                        for j in range(kt):
                            oh = opool.tile([128, 512], BF16, tag="oh")
                            nc.vector.tensor_scalar(
                                oh[:], iota[:], ds[:, n - n0:n - n0 + 1],
                                rs[:, n - n0:n - n0 + 1], ALU.is_equal,
                                ALU.mult)
                            for c in range(4):
                                nc.tensor.matmul(
                                    ps[:],
                                    feat[:, n - n0, 64 * c:64 * c + 64],
                                    oh[:, 128 * c:128 * c + 128],
                                    start=(j == 0 and c == 0),
                                    stop=(j == kt - 1 and c == 3))
                            n += 1
                        if t % 4 == 0:
                            mt4 = mpool.tile([64, 512], BF16, tag="mt")
                        nc.vector.tensor_copy(
                            mt4[:, (t % 4) * 128:(t % 4) * 128 + 128], ps[:])
                        if t % 4 == 3 or t == NTILE - 1:
                            w = (t % 4 + 1) * 128
                            tb = t - t % 4
                            ps2 = psypool.tile([128, 512], F32, tag="y")
                            nc.tensor.matmul(ps2[:, :w], wdir[name][:],
                                             mt4[:, :w], start=True,
                                             stop=True)
                            yt = epool.tile([128, 512], F32, tag="yt")
                            emit_elu(ps2[:, :w], bdir[name][:], yt[:, :w], w,
                                     epool, "d")
                            nc.sync.dma_start(
                                d["yT"][:, tb * 128:tb * 128 + w], yt[:, :w])
                    t0 = t1

            # ---------------- self branch ----------------
            for ch in range(-(-NPAD // 512)):
                sw = min(512, NPAD - 512 * ch)
                xT = spool.tile([128, 512], BF16, tag="xT")
                nc.scalar.dma_start_transpose(
                    xT[:, :sw], xself_d[512 * ch:512 * ch + sw, :])
                ps2 = psypool.tile([128, 512], F32, tag="y")
                for k in range(4):
                    ps1 = pshpool.tile([128, 512], F32, tag="h")
                    nc.tensor.matmul(ps1[:, :sw],
                                     w1[:, 128 * k:128 * k + 128],
                                     xT[0:64, :sw], start=True, stop=True)
                    hk = spool.tile([128, 512], BF16, tag="hk")
                    emit_elu(ps1[:, :sw], b1[:, k:k + 1], hk[:, :sw], sw,
                             spool, "s")
                    nc.tensor.matmul(ps2[:, :sw], w2[:, 128 * k:128 * k + 128],
                                     hk[:, :sw], start=(k == 0), stop=(k == 3))
                yt = spool.tile([128, 512], F32, tag="yts")
                emit_elu(ps2[:, :sw], b2[:], yt[:, :sw], sw, spool, "o")
                nc.sync.dma_start(yself_d[:, 512 * ch:512 * ch + sw],
                                  yt[:, :sw])

    nc.compile()
    return nc


def run(inputs, trace=False):
    x = np.asarray(inputs["x"], np.float32)
    ei = np.asarray(inputs["edge_index"], np.int64)
    src, dst = ei[0], ei[1]

    gin, din, rin, kt_in = prep_direction(dst, src)
    gout, dout, rout, kt_out = prep_direction(src, dst)

    x4 = np.zeros((NQROW, 256), np.float32)
    x4[:N // 4] = x.reshape(N // 4, 256)
    x4b = x4.astype(ml_dtypes.bfloat16)
    xsb = np.zeros((NCORES, NPAD, 128), np.float32)
    for c in range(NCORES):
        take = min(NPC, N - c * NPC)
        xsb[c, :take, :64] = x[c * NPC:c * NPC + take]
    xsb16 = xsb.astype(ml_dtypes.bfloat16)

    def bf(a):
        return np.ascontiguousarray(np.asarray(a, np.float32)).astype(
            ml_dtypes.bfloat16)

    iota_shift = np.arange(512, dtype=np.float32) - 256.0
    iota512 = np.tile(iota_shift, (128, 1)).astype(ml_dtypes.bfloat16)

    W1 = np.asarray(inputs["W1"], np.float32)
    b1 = np.asarray(inputs["b1"], np.float32)
    W2 = np.asarray(inputs["W2"], np.float32)
    b2 = np.asarray(inputs["b2"], np.float32)
    w2p = np.zeros((128, 512), np.float32)
    for k in range(4):
        w2p[:, 128 * k:128 * (k + 1)] = W2[128 * k:128 * (k + 1), :]

    nc = build_nc(kt_in, kt_out)

    in_maps = []
    for c in range(NCORES):
        in_maps.append({
            "x4b": x4b,
            "xself": xsb16[c],
            "iota512": iota512,
            "w1": bf(W1), "b1": np.ascontiguousarray(b1.reshape(4, 128).T),
            "w2p": bf(w2p), "b2": b2.reshape(128, 1),
            "gidx_in": gin[c], "dsel_in": din[c], "rsel_in": rin[c],
            "w_in": bf(inputs["W_in"]),
            "b_in": np.asarray(inputs["b_in"], np.float32).reshape(128, 1),
            "gidx_out": gout[c], "dsel_out": dout[c], "rsel_out": rout[c],
            "w_out": bf(inputs["W_out"]),
            "b_out": np.asarray(inputs["b_out"], np.float32).reshape(128, 1),
        })

    kw = {}
    if trace:
        kw = dict(trace=True, trace_cores=[0])
    res = run_bass_kernel_spmd(nc, in_maps, core_ids=list(range(NCORES)),
                               **kw)

    def gather_out(nm):
        return np.concatenate(
            [res.results[c][nm][:, :NPC].T for c in range(NCORES)], 0)

    return (gather_out("yT_in"), gather_out("yT_out"),
            gather_out("yT_self")), res


def kernel(**inputs):
    (x_in, x_out, x_self), _ = run(inputs, trace=False)
    return x_in, x_out, x_self
